# revision 1
# baseline (speedup 1.0000x reference)
"""Causal self-attention (B=2, T=2048, C=1024, H=16, D=64) on 8 TRN2 cores.

Sharding: core c handles batch b=c//4 and head group g=c%4 (heads 4g..4g+3).
Per core, on device (all matmuls bf16, fp32 PSUM accumulation):
  - qkv projection from pre-transposed x^T (host-prepped, bf16):
      qT/kT in transposed layout [d, m] as head-pairs [128, T];
      V in natural layout [m, d] for the core's 4 heads, with a ones column
      per head appended so the attn@V matmul also produces the softmax
      denominator (row 64 of the [65, m] PSUM output).
  - scores computed transposed S^T[j, m] = kT.T @ qT (1/8 scale folded into
    wq on host); softmax WITHOUT max subtraction (scores are O(5), exact in
    fp32); exp on ScalarE straight out of PSUM; causal handled by skipping
    fully-masked blocks and one affine_select on each diagonal block.
  - y^T normalized via a K=1 broadcast matmul of 1/l, stays transposed as
    the c_proj stationary; c_proj partial output [T, C] in fp32.
Host: sums the 4 partials per batch, adds b_proj and the v-bias term.
"""

import math
import numpy as np
import ml_dtypes

import concourse.bass as bass
import concourse.tile as tile
from concourse import bacc, mybir
from concourse.bass_utils import run_bass_kernel_spmd

BF16 = ml_dtypes.bfloat16
F32 = mybir.dt.float32
BF = mybir.dt.bfloat16

B, T, C = 2, 2048, 1024
H, D = 16, 64
N_CORES = 8
GROUPS = 4            # head groups (cores per batch)
HPC = 4               # heads per core
CC = 8                # contraction chunks: C / 128
MB = T // 128         # 16 m-blocks
MC = T // 512         # 4 m-chunks


DEFAULT_OPTS = dict(
    stages=("qkv", "attn", "cproj"),
    qkv_bufs=3, s_bufs=3, y_bufs=2, bc_bufs=1, o_bufs=1,
    expS_bufs=8, out_bufs=3, const_bufs=1, work_bufs=1,
    bcast_engine="vector", out_copy_engine="vector",
)


def emit_body(nc, tc, ctx_pools, xt_ap, wqk_ap, wv_ap, wp_ap, bqk_ap, outp_ap,
              opts=DEFAULT_OPTS):
    (const_pool, qkT_pool, v_pool, yT_pool, expS_pool, out_pool,
     recip_pool, bcast_pool) = ctx_pools

    # per-chunk tiles so compute can start as soon as each chunk's DMA lands
    # spread loads over SP-HWDGE, ACT-HWDGE and Pool-SWDGE queues
    xt, wqk, wv = [], [], []
    for cc in range(CC):
        xtc = const_pool.tile([128, T], BF, tag=f"xt{cc}")
        (nc.sync if cc % 2 == 0 else nc.scalar).dma_start(
            xtc[:], xt_ap[cc * 128:(cc + 1) * 128, :])
        xt.append(xtc)
        wqc = const_pool.tile([128, 512], BF, tag=f"wqk{cc}")
        (nc.sync if cc % 2 == 1 else nc.scalar).dma_start(
            wqc[:], wqk_ap[cc * 128:(cc + 1) * 128, :])
        wqk.append(wqc)
        wvc = const_pool.tile([128, 256], BF, tag=f"wv{cc}")
        (nc.sync if cc % 2 == 1 else nc.scalar).dma_start(
            wvc[:], wv_ap[cc * 128:(cc + 1) * 128, :])
        wv.append(wvc)
    wp = const_pool.tile([128, 2 * 1024], BF, tag="wp")
    for p2 in range(2):
        nc.sync.dma_start(wp[:, p2 * 1024:(p2 + 1) * 1024], wp_ap[p2 * 128:(p2 + 1) * 128, :])
    bqk = const_pool.tile([128, 4], F32, tag="bqk")
    nc.sync.dma_start(bqk[:], bqk_ap[:])
    ones = const_pool.tile([1, 64], BF, tag="ones")
    nc.gpsimd.memset(ones[:], 1.0)
    # lower-triangular (keep j<=m) bf16 mask for diagonal score blocks
    cmask = const_pool.tile([128, 128], BF, tag="cmask")
    nc.gpsimd.memset(cmask[:], 1.0)
    nc.gpsimd.affine_select(
        out=cmask[:], in_=cmask[:], compare_op=mybir.AluOpType.is_ge,
        fill=0.0, base=0, pattern=[[1, 128]], channel_multiplier=-1)

    qkT = qkT_pool.tile([128, 4 * T], BF, tag="qkT")   # q pair0, q pair1, k pair0, k pair1
    v_all = v_pool.tile([128, MB * 260], BF, tag="v")  # per m-block: 4x(64 v cols + ones col)
    yT = yT_pool.tile([128, 2 * T], BF, tag="yT")      # per pair: [hd, m]

    # ---- qkv projection ----
    if "qkv" not in opts["stages"]:
        return
    with tc.tile_pool(name="qkv_ps", bufs=opts["qkv_bufs"], space="PSUM") as qkv_psum:
        for pp in range(4):          # q0 q1 k0 k1 pair outputs
            for mc in range(MC):
                ps = qkv_psum.tile([128, 512], F32, tag="qkvps")
                for cc in range(CC):
                    nc.tensor.matmul(
                        ps[:],
                        lhsT=wqk[cc][:, pp * 128:(pp + 1) * 128],
                        rhs=xt[cc][:, mc * 512:(mc + 1) * 512],
                        start=(cc == 0), stop=(cc == CC - 1))
                nc.vector.tensor_scalar_add(
                    qkT[:, pp * T + mc * 512: pp * T + (mc + 1) * 512],
                    ps[:], bqk[:, pp:pp + 1])
        for mb in range(MB):
            ps = qkv_psum.tile([128, 512], F32, tag="qkvps")
            for cc in range(CC):
                nc.tensor.matmul(
                    ps[:, 0:256],
                    lhsT=xt[cc][:, mb * 128:(mb + 1) * 128],
                    rhs=wv[cc][:],
                    start=(cc == 0), stop=(cc == CC - 1))
            vb = v_all[:, mb * 260:(mb + 1) * 260]
            nc.gpsimd.memset(vb.rearrange("p (h x) -> p h x", x=65)[:, :, 64:65], 1.0)
            nc.vector.tensor_copy(
                vb.rearrange("p (h x) -> p h x", x=65)[:, :, 0:64],
                ps[:, 0:256].rearrange("p (h x) -> p h x", x=64))

    # ---- attention (m-chunk outer, head inner) with c_proj inlined ----
    if "attn" not in opts["stages"]:
        return
    do_cproj = "cproj" in opts["stages"]
    with (
        tc.tile_pool(name="s_ps", bufs=opts["s_bufs"], space="PSUM") as s_psum,
        tc.tile_pool(name="y_ps", bufs=opts["y_bufs"], space="PSUM") as y_psum,
        tc.tile_pool(name="bc_ps", bufs=opts["bc_bufs"], space="PSUM") as bc_psum,
        tc.tile_pool(name="o_ps", bufs=opts["o_bufs"], space="PSUM") as o_psum,
    ):
        for mc in range(MC):
            for h in range(HPC):
                pp, half = h // 2, h % 2
                prow = 64 * half
                qoff = pp * T
                koff = (2 + pp) * T
                yp = y_psum.tile([65, 512], F32, tag="ypsum")
                last_jb = 4 * mc + 3
                for jb in range(4 * mc + 4):
                    off = max(0, (jb - 4 * mc) * 128)
                    w = 512 - off
                    m_abs = mc * 512 + off
                    sp = s_psum.tile([128, 512], F32, tag="spsum")
                    nc.tensor.matmul(
                        sp[:, 0:w],
                        lhsT=qkT[prow:prow + 64, koff + jb * 128: koff + (jb + 1) * 128],
                        rhs=qkT[prow:prow + 64, qoff + m_abs: qoff + m_abs + w],
                        start=True, stop=True)
                    es = expS_pool.tile([128, 512], BF, tag="expS")
                    nc.scalar.activation(es[:, 0:w], sp[:, 0:w],
                                         mybir.ActivationFunctionType.Exp)
                    if jb >= 4 * mc:
                        nc.vector.tensor_mul(es[:, 0:128], es[:, 0:128], cmask[:])
                    nc.tensor.matmul(
                        yp[:, off:512],
                        lhsT=v_all[:, jb * 260 + h * 65: jb * 260 + (h + 1) * 65],
                        rhs=es[:, 0:w],
                        start=(jb == 0), stop=(jb == last_jb))
                rc = recip_pool.tile([1, 512], BF, tag="recip")
                with nc.allow_low_precision(reason="1/l broadcastee; bf16 ok"):
                    nc.vector.reciprocal(rc[:], yp[64:65, :])
                bc = bc_psum.tile([64, 512], F32, tag="bcps")
                nc.tensor.matmul(bc[:], lhsT=ones[:], rhs=rc[:], start=True, stop=True)
                bs = bcast_pool.tile([64, 512], F32, tag="bcsb")
                if opts["bcast_engine"] == "scalar":
                    nc.scalar.activation(bs[:], bc[:], mybir.ActivationFunctionType.Copy)
                else:
                    nc.vector.tensor_copy(bs[:], bc[:])
                nc.vector.tensor_mul(
                    yT[prow:prow + 64, pp * T + mc * 512: pp * T + (mc + 1) * 512],
                    yp[0:64, :], bs[:])
            if not do_cproj:
                continue
            for mb in range(4 * mc, 4 * mc + 4):
                op = o_psum.tile([128, 1024], F32, tag="opsum")
                for pp2 in range(2):
                    for nch in range(2):
                        nc.tensor.matmul(
                            op[:, nch * 512:(nch + 1) * 512],
                            lhsT=yT[:, pp2 * T + mb * 128: pp2 * T + (mb + 1) * 128],
                            rhs=wp[:, pp2 * 1024 + nch * 512: pp2 * 1024 + (nch + 1) * 512],
                            start=(pp2 == 0), stop=(pp2 == 1))
                ob = out_pool.tile([128, 1024], F32, tag="outsb")
                if opts["out_copy_engine"] == "vector":
                    nc.vector.tensor_copy(ob[:], op[:])
                else:
                    nc.scalar.activation(ob[:], op[:], mybir.ActivationFunctionType.Copy)
                nc.sync.dma_start(outp_ap[mb * 128:(mb + 1) * 128, :], ob[:])


def build(reps=1, opts=None):
    opts = {**DEFAULT_OPTS, **(opts or {})}
    nc = bacc.Bacc("TRN2", target_bir_lowering=False, debug=False)
    xt_ap = nc.dram_tensor("xt", [C, T], BF, kind="ExternalInput").ap()
    wqk_ap = nc.dram_tensor("wqk", [C, 512], BF, kind="ExternalInput").ap()
    wv_ap = nc.dram_tensor("wv", [C, 256], BF, kind="ExternalInput").ap()
    wp_ap = nc.dram_tensor("wp", [256, 1024], BF, kind="ExternalInput").ap()
    bqk_ap = nc.dram_tensor("bqk", [128, 4], F32, kind="ExternalInput").ap()
    outp_ap = nc.dram_tensor("outp", [T, C], F32, kind="ExternalOutput").ap()

    with tile.TileContext(nc) as tc:
        with (
            tc.tile_pool(name="const", bufs=opts["const_bufs"]) as const_pool,
            tc.tile_pool(name="qkT", bufs=opts["work_bufs"]) as qkT_pool,
            tc.tile_pool(name="v", bufs=opts["work_bufs"]) as v_pool,
            tc.tile_pool(name="yT", bufs=opts["work_bufs"]) as yT_pool,
            tc.tile_pool(name="expS", bufs=opts["expS_bufs"]) as expS_pool,
            tc.tile_pool(name="outsb", bufs=opts["out_bufs"]) as out_pool,
            tc.tile_pool(name="recip", bufs=2) as recip_pool,
            tc.tile_pool(name="bcast", bufs=2) as bcast_pool,
        ):
            pools = (const_pool, qkT_pool, v_pool, yT_pool, expS_pool,
                     out_pool, recip_pool, bcast_pool)
            for _ in range(reps):
                emit_body(nc, tc, pools, xt_ap, wqk_ap, wv_ap, wp_ap, bqk_ap, outp_ap, opts)
    nc.compile()
    return nc


def build_looped(n_iters, opts=None):
    """Body wrapped in a hardware For_i loop, for wall-clock slope timing."""
    opts = {**DEFAULT_OPTS, **(opts or {})}
    nc = bacc.Bacc("TRN2", target_bir_lowering=False, debug=False)
    xt_ap = nc.dram_tensor("xt", [C, T], BF, kind="ExternalInput").ap()
    wqk_ap = nc.dram_tensor("wqk", [C, 512], BF, kind="ExternalInput").ap()
    wv_ap = nc.dram_tensor("wv", [C, 256], BF, kind="ExternalInput").ap()
    wp_ap = nc.dram_tensor("wp", [256, 1024], BF, kind="ExternalInput").ap()
    bqk_ap = nc.dram_tensor("bqk", [128, 4], F32, kind="ExternalInput").ap()
    outp_ap = nc.dram_tensor("outp", [T, C], F32, kind="ExternalOutput").ap()
    with tile.TileContext(nc) as tc:
        with (
            tc.tile_pool(name="const", bufs=opts["const_bufs"]) as const_pool,
            tc.tile_pool(name="qkT", bufs=opts["work_bufs"]) as qkT_pool,
            tc.tile_pool(name="v", bufs=opts["work_bufs"]) as v_pool,
            tc.tile_pool(name="yT", bufs=opts["work_bufs"]) as yT_pool,
            tc.tile_pool(name="expS", bufs=opts["expS_bufs"]) as expS_pool,
            tc.tile_pool(name="outsb", bufs=opts["out_bufs"]) as out_pool,
            tc.tile_pool(name="recip", bufs=2) as recip_pool,
            tc.tile_pool(name="bcast", bufs=2) as bcast_pool,
        ):
            pools = (const_pool, qkT_pool, v_pool, yT_pool, expS_pool,
                     out_pool, recip_pool, bcast_pool)
            with tc.For_i(0, n_iters, 1):
                emit_body(nc, tc, pools, xt_ap, wqk_ap, wv_ap, wp_ap, bqk_ap,
                          outp_ap, opts)
    nc.compile()
    return nc


_NC_CACHE = {}


def _get_nc(reps=1, opts=None):
    key = (reps, tuple(sorted((opts or {}).items())))
    if key not in _NC_CACHE:
        _NC_CACHE[key] = build(reps, opts)
    return _NC_CACHE[key]


def make_in_maps(x, w_attn, b_attn, w_proj):
    x = np.asarray(x, np.float32)
    w_attn = np.asarray(w_attn, np.float32)
    b_attn = np.asarray(b_attn, np.float32)
    in_maps = []
    xt_b = [np.ascontiguousarray(x[b].T).astype(BF16) for b in range(B)]
    for c in range(N_CORES):
        b, g = divmod(c, GROUPS)
        h0 = HPC * g
        qs, ks = h0 * D, C + h0 * D
        wqk = np.concatenate([
            0.125 * w_attn[:, qs:qs + 128], 0.125 * w_attn[:, qs + 128:qs + 256],
            w_attn[:, ks:ks + 128], w_attn[:, ks + 128:ks + 256]], axis=1).astype(BF16)
        wv = w_attn[:, 2 * C + g * 256: 2 * C + (g + 1) * 256].astype(BF16)
        wp = np.asarray(w_proj, np.float32)[g * 256:(g + 1) * 256, :].astype(BF16)
        bqk = np.stack([
            0.125 * b_attn[qs:qs + 128], 0.125 * b_attn[qs + 128:qs + 256],
            b_attn[ks:ks + 128], b_attn[ks + 128:ks + 256]], axis=1).astype(np.float32)
        in_maps.append({"xt": xt_b[b], "wqk": np.ascontiguousarray(wqk),
                        "wv": np.ascontiguousarray(wv), "wp": np.ascontiguousarray(wp),
                        "bqk": np.ascontiguousarray(bqk)})
    return in_maps


def assemble_output(results, b_attn, w_proj, b_proj):
    b_attn = np.asarray(b_attn, np.float32)
    w_proj = np.asarray(w_proj, np.float32)
    b_proj = np.asarray(b_proj, np.float32)
    extra = b_attn[2 * C:] @ w_proj + b_proj  # v-bias flows through softmax as +bv
    out = np.empty((B, T, C), np.float32)
    for b in range(B):
        acc = results[4 * b]["outp"].astype(np.float32).copy()
        for g in range(1, GROUPS):
            acc += results[4 * b + g]["outp"]
        out[b] = acc + extra
    return out


def kernel(x, w_attn, b_attn, w_proj, b_proj):
    nc = _get_nc(reps=1)
    in_maps = make_in_maps(x, w_attn, b_attn, w_proj)
    res = run_bass_kernel_spmd(nc, in_maps, list(range(N_CORES)))
    return assemble_output(res.results, b_attn, w_proj, b_proj)



# revision 16
# speedup vs baseline: 1.7013x; 1.7013x over previous
"""Causal self-attention (B=2, T=2048, C=1024, H=16, D=64) on 8 TRN2 cores.

Sharding: core c handles batch b=c//4 and head group g=c%4 (heads 4g..4g+3).

Per core (all matmuls bf16, fp32 PSUM):
  - host packs inputs so each SBUF tensor loads in O(1) DMAs:
      xt:  [4 mc][8 cc][128 p][512 m]   (x^T m-chunk-major)
      wqk: [4 pp][8 cc][128 p][128 n]   (q pair0/1, k pair0/1; 1/8 folded into q)
      wv:  [8 cc][128 p][256 n]
      wp:  [2 pp2][128 p][1024 n]
  - per m-chunk mc: qkv projection for that chunk, then attention for the
    chunk's 4 heads. Scores computed transposed S^T[j, m] = kT.T @ qT;
    softmax without max subtraction (exact in fp32); exp on ScalarE from
    PSUM; causal = skip fully-masked blocks + bf16 mask-mul on diagonal
    blocks. attn@V appends a ones column per head so row 64 of the [65, m]
    PSUM output is the softmax denominator. Normalize: broadcast l via a
    K=1 ones matmul, reciprocal on DVE straight into the broadcast shape.
  - software pipelining: S-matmuls lead attn@V by `lookahead` blocks so PE
    never waits on the ScalarE exp round trip; qkv(mc+1) and cproj(mc-1)
    groups are emitted as PE filler inside the ACT-bound attention windows.
  - c_proj partial output stored bf16; host sums the 4 partials per batch,
    adds b_proj and the v-bias term in fp32.
"""

import math
from collections import deque

import numpy as np
import ml_dtypes

import concourse.bass as bass
import concourse.tile as tile
from concourse import bacc, mybir
from concourse.bass_utils import run_bass_kernel_spmd

BF16 = ml_dtypes.bfloat16
F32 = mybir.dt.float32
BF = mybir.dt.bfloat16

B, T, C = 2, 2048, 1024
H, D = 16, 64
N_CORES = 8
GROUPS = 4            # head groups (cores per batch)
HPC = 4               # heads per core
CC = 8                # contraction chunks: C / 128
MB = T // 128         # 16 m-blocks
MC = T // 512         # 4 m-chunks


DEFAULT_OPTS = dict(
    stages=("qkv", "attn", "cproj"),
    s_bufs=2, yp_bufs=2, acc_bufs=2,
    es_bufs=4, out_bufs=2, bs_bufs=2,
    warmup=10,
    max_mc=MC,
    exp_light=False,
    av_lag=2,
    dbuf=1,
    out_dt="bf16",
    hint_engines=(),
    staggered_reset=False,
)


def emit_body(nc, tc, pools, aps, opts):
    (const_pool, qkT_pool, v_pool, yT_pool, es_pool, out_pool,
     lrow_pool, bs_pool, psum_pool) = pools
    xt_ap, wqk_ap, wv_ap, wp_ap, bqk_ap, outp_ap = aps
    do_attn = "attn" in opts["stages"]
    do_cproj = "cproj" in opts["stages"] and do_attn
    out_dt = BF if opts["out_dt"] == "bf16" else F32

    # ---- input DMAs: host arrays are the exact SBUF layout (contiguous
    # multi-KB runs per partition); spread across SP and ACT HWDGE queues ----
    db = opts["dbuf"]
    bqk = const_pool.tile([128, 4], F32, tag="bqk", bufs=db)
    nc.sync.dma_start(bqk[:], bqk_ap[:])
    wqk = const_pool.tile([128, 4096], BF, tag="wqk", bufs=db)
    xt = []
    for mc in range(MC):
        xtc = const_pool.tile([128, 4096], BF, tag=f"xt{mc}", bufs=db)
        xt.append(xtc)
    wv = const_pool.tile([128, 2048], BF, tag="wv", bufs=db)
    wp = const_pool.tile([128, 2048], BF, tag="wp", bufs=db)
    # q pair halves of wqk first (rows: pp-major), k pairs next
    nc.sync.dma_start(wqk[:, 0:2048].rearrange("p (q n) -> p q n", q=2),
                      wqk_ap[0:256, :].rearrange("(q p) n -> p q n", p=128))
    nc.scalar.dma_start(xt[0][:], xt_ap[0:128, :])
    nc.sync.dma_start(wqk[:, 2048:4096].rearrange("p (q n) -> p q n", q=2),
                      wqk_ap[256:512, :].rearrange("(q p) n -> p q n", p=128))
    nc.scalar.dma_start(wv[:], wv_ap[:])
    nc.sync.dma_start(xt[1][:], xt_ap[128:256, :])
    nc.scalar.dma_start(xt[2][:], xt_ap[256:384, :])
    nc.sync.dma_start(xt[3][:], xt_ap[384:512, :])
    nc.scalar.dma_start(wp[:], wp_ap[:])

    # ---- small consts ----
    # lower-triangular (keep j<=m) bf16 mask for diagonal score blocks
    cmask = const_pool.tile([128, 128], BF, tag="cmask", bufs=db)
    nc.gpsimd.memset(cmask[:], 1.0)
    nc.gpsimd.affine_select(
        out=cmask[:], in_=cmask[:], compare_op=mybir.AluOpType.is_ge,
        fill=0.0, base=0, pattern=[[1, 128]], channel_multiplier=-1)

    ones = const_pool.tile([1, 64], BF, tag="ones", bufs=db)
    nc.gpsimd.memset(ones[:], 1.0)
    cmask8 = const_pool.tile([128, 1024], BF, tag="cmask8")
    if opts["exp_light"]:
        nc.gpsimd.memset(cmask8[:], 0.01)

    qkT = qkT_pool.tile([128, 4 * T], BF, tag="qkT", bufs=db)   # q pair0, q pair1, k pair0, k pair1
    v_all = v_pool.tile([128, MB * 260], BF, tag="v", bufs=db)  # per m-block: 4x(64 v cols + ones col)
    yT = yT_pool.tile([128, 2 * T], BF, tag="yT", bufs=db)      # per pair: [hd, m]

    # PE warmup during the input-DMA head: junk matmuls on last iteration's
    # qkT keep the HAM activity window busy so real matmuls start at 2.4GHz
    for wi in range(opts["warmup"]):
        wacc = psum_pool.tile([128, 512], F32, tag="acc", bufs=opts["acc_bufs"],
                              name=f"warm{wi}")
        nc.tensor.matmul(wacc[:], lhsT=qkT[:, 0:128], rhs=qkT[:, 1024:1536],
                         start=True, stop=True)

    # ---- filler group emitters (PE work without ACT deps) ----
    def qkv_groups(mc):
        def qk_group(pp):
            def go():
                acc = psum_pool.tile([128, 512], F32, tag="acc", bufs=opts["acc_bufs"],
                                     name=f"acc_qk{mc}_{pp}")
                for cc in range(CC):
                    nc.tensor.matmul(
                        acc[:],
                        lhsT=wqk[:, pp * 1024 + cc * 128: pp * 1024 + (cc + 1) * 128],
                        rhs=xt[mc][:, cc * 512:(cc + 1) * 512],
                        start=(cc == 0), stop=(cc == CC - 1))
                nc.vector.tensor_scalar_add(
                    qkT[:, pp * T + mc * 512: pp * T + (mc + 1) * 512],
                    acc[:], bqk[:, pp:pp + 1])
            return go

        def v_group(mbl):
            mb = 4 * mc + mbl
            def go():
                acc = psum_pool.tile([128, 512], F32, tag="acc", bufs=opts["acc_bufs"],
                                     name=f"acc_v{mb}")
                for cc in range(CC):
                    nc.tensor.matmul(
                        acc[:, 0:256],
                        lhsT=xt[mc][:, cc * 512 + mbl * 128: cc * 512 + (mbl + 1) * 128],
                        rhs=wv[:, cc * 256:(cc + 1) * 256],
                        start=(cc == 0), stop=(cc == CC - 1))
                vb = v_all[:, mb * 260:(mb + 1) * 260]
                nc.gpsimd.memset(vb.rearrange("p (h x) -> p h x", x=65)[:, :, 64:65], 1.0)
                nc.vector.tensor_copy(
                    vb.rearrange("p (h x) -> p h x", x=65)[:, :, 0:64],
                    acc[:, 0:256].rearrange("p (h x) -> p h x", x=64))
            return go

        return [qk_group(0), qk_group(2), qk_group(1), qk_group(3),
                v_group(0), v_group(1), v_group(2), v_group(3)]

    def cproj_groups(mc):
        """Per (mb, nch): an A closure (pp2=0 matmul — needs only pair-0 yT)
        and a B closure (pp2=1 matmul + psum drain + out DMA)."""
        a_closures, b_closures = [], []
        for mbl in range(4):
            mb = 4 * mc + mbl
            ob = [None]
            for nch in range(2):
                st = {}
                def ca(mb=mb, nch=nch, st=st):
                    acc = psum_pool.tile([128, 512], F32, tag="acc", bufs=opts["acc_bufs"],
                                         name=f"acc_o{mb}_{nch}")
                    nc.tensor.matmul(
                        acc[:],
                        lhsT=yT[:, mb * 128:(mb + 1) * 128],
                        rhs=wp[:, nch * 512:(nch + 1) * 512],
                        start=True, stop=False)
                    st["acc"] = acc
                def cb(mb=mb, nch=nch, st=st, ob=ob):
                    acc = st["acc"]
                    nc.tensor.matmul(
                        acc[:],
                        lhsT=yT[:, T + mb * 128: T + (mb + 1) * 128],
                        rhs=wp[:, 1024 + nch * 512: 1024 + (nch + 1) * 512],
                        start=False, stop=True)
                    if nch == 0:
                        ob[0] = out_pool.tile([128, 1024], out_dt, tag="outsb",
                                              bufs=opts["out_bufs"], name=f"outsb{mb}")
                    nc.vector.tensor_copy(ob[0][:, nch * 512:(nch + 1) * 512], acc[:])
                    if nch == 1:
                        nc.sync.dma_start(outp_ap[mb * 128:(mb + 1) * 128, :], ob[0][:])
                a_closures.append(ca)
                b_closures.append(cb)
        return a_closures, b_closures

    if not do_attn:
        # decomposition mode: just run qkv chunks serially
        for mc in range(MC):
            for g in qkv_groups(mc):
                g()
        return

    # window 0 prologue: qkv(0) with no attention to overlap
    for g in qkv_groups(0):
        g()

    for mc in range(min(MC, opts["max_mc"])):
        fillers = deque()
        if mc + 1 < MC:
            fillers.extend(qkv_groups(mc + 1))
        if do_cproj and mc >= 1:
            ca, cb = cproj_groups(mc - 1)
            fillers.extend(x for pair in zip(ca, cb) for x in pair)
        nblk = 4 * mc + 4
        total_slots = 2 * (nblk + 1)  # approximate; av_lag shifts slightly
        pace = max(1, (total_slots * 10) // max(1, len(fillers)))
        slot = 0

        def tick():
            nonlocal slot
            slot += 10
            while fillers and slot >= pace:
                slot -= pace
                fillers.popleft()()

        # heads processed in half-alternating pairs (2pi, 2pi+1): their S
        # matmuls use disjoint PE row halves and run concurrently; both
        # heads' scores share one 2-bank psum tile -> single [128,1024] exp
        for pi in range(2):
            pp = pi
            qoff = pp * T
            koff = (2 + pp) * T
            ypA = psum_pool.tile([65, 512], F32, tag="yp", bufs=opts["yp_bufs"],
                                 name=f"ypA{mc}_{pi}")
            ypB = psum_pool.tile([65, 512], F32, tag="yp", bufs=opts["yp_bufs"],
                                 name=f"ypB{mc}_{pi}")
            hA, hB = 2 * pi, 2 * pi + 1
            LAG = min(opts["av_lag"], nblk)
            stiles = {}
            for t in range(nblk + LAG):
                if t < nblk:
                    jb = t
                    off = max(0, (jb - 4 * mc) * 128)
                    w = 512 - off
                    m_abs = mc * 512 + off
                    s2 = psum_pool.tile([128, 1024], F32, tag="s", bufs=opts["s_bufs"],
                                        name=f"s{mc}_{pi}_{jb}")
                    nc.tensor.matmul(
                        s2[:, 0:w],
                        lhsT=qkT[0:64, koff + jb * 128: koff + (jb + 1) * 128],
                        rhs=qkT[0:64, qoff + m_abs: qoff + m_abs + w],
                        start=True, stop=True)
                    nc.tensor.matmul(
                        s2[:, 512:512 + w],
                        lhsT=qkT[64:128, koff + jb * 128: koff + (jb + 1) * 128],
                        rhs=qkT[64:128, qoff + m_abs: qoff + m_abs + w],
                        start=True, stop=True)
                    es2 = es_pool.tile([128, 1024], BF, tag="expS", bufs=opts["es_bufs"],
                                       name=f"es{mc}_{pi}_{jb}")
                    if opts["exp_light"]:
                        # timing-debug only: stand-in writer for es2, wrong values
                        nc.vector.tensor_copy(es2[:, 0:1024], cmask8[:])
                    elif w == 512:
                        nc.scalar.activation(es2[:, 0:1024], s2[:, 0:1024],
                                             mybir.ActivationFunctionType.Exp)
                    else:
                        nc.scalar.activation(
                            es2[:].rearrange("p (g x) -> p g x", g=2)[:, :, 0:w],
                            s2[:].rearrange("p (g x) -> p g x", g=2)[:, :, 0:w],
                            mybir.ActivationFunctionType.Exp)
                    if jb >= 4 * mc:
                        nc.gpsimd.tensor_mul(es2[:, 0:128], es2[:, 0:128], cmask[:])
                        nc.gpsimd.tensor_mul(es2[:, 512:640], es2[:, 512:640], cmask[:])
                    stiles[jb] = (es2, off, w)
                if t >= LAG:
                    jb = t - LAG
                    es2, off, w = stiles.pop(jb)
                    nc.tensor.matmul(
                        ypA[:, off:512],
                        lhsT=v_all[:, jb * 260 + hA * 65: jb * 260 + (hA + 1) * 65],
                        rhs=es2[:, 0:w],
                        start=(jb == 0), stop=(jb == nblk - 1))
                    nc.tensor.matmul(
                        ypB[:, off:512],
                        lhsT=v_all[:, jb * 260 + hB * 65: jb * 260 + (hB + 1) * 65],
                        rhs=es2[:, 512:512 + w],
                        start=(jb == 0), stop=(jb == nblk - 1))
                tick()
            # normalize both heads: l rows -> ones-matmul broadcasts into the
            # two partition halves of one psum block -> single 1/l on DVE
            lrowA = lrow_pool.tile([1, 512], BF, tag="lrow", bufs=4,
                                   name=f"lrowA{mc}_{pi}")
            lrowB = lrow_pool.tile([1, 512], BF, tag="lrow", bufs=4,
                                   name=f"lrowB{mc}_{pi}")
            with nc.allow_low_precision(reason="softmax denom; bf16 ok"):
                nc.vector.tensor_copy(lrowA[:], ypA[64:65, :])
                nc.vector.tensor_copy(lrowB[:], ypB[64:65, :])
            bc2 = psum_pool.tile([128, 512], F32, tag="acc", bufs=opts["acc_bufs"],
                                 name=f"bc{mc}_{pi}")
            nc.tensor.matmul(bc2[0:64, :], lhsT=ones[:], rhs=lrowA[:],
                             start=True, stop=True)
            nc.tensor.matmul(bc2[64:128, :], lhsT=ones[:], rhs=lrowB[:],
                             start=True, stop=True)
            bs2 = bs_pool.tile([128, 512], F32, tag="bs", bufs=opts["bs_bufs"],
                               name=f"bs{mc}_{pi}")
            # single-pass approx (~18 bits) instead of the 8-iteration HW
            # divide — denominators are well inside fp32 normal range
            nc.vector.reciprocal_approx_fast(bs2[:], bc2[:])
            nc.vector.tensor_mul(
                yT[0:64, pp * T + mc * 512: pp * T + (mc + 1) * 512],
                ypA[0:64, :], bs2[0:64, :])
            nc.vector.tensor_mul(
                yT[64:128, pp * T + mc * 512: pp * T + (mc + 1) * 512],
                ypB[0:64, :], bs2[64:128, :])
        while fillers:
            fillers.popleft()()

    if opts["max_mc"] < MC:
        return
    if do_cproj:
        ca, cb = cproj_groups(MC - 1)
        for g in ca:
            g()
        for g in cb:
            g()


def _make_aps(nc):
    # host-packed SBUF images: xt [4 mc x 128 p, 4096], wqk [4 pp... as
    # [2 qk x 128 p, 2048], wv [128 p, 2048], wp [128 p, 2048]
    xt_ap = nc.dram_tensor("xt", [512, 4096], BF, kind="ExternalInput").ap()
    wqk_ap = nc.dram_tensor("wqk", [512, 1024], BF, kind="ExternalInput").ap()
    wv_ap = nc.dram_tensor("wv", [128, 2048], BF, kind="ExternalInput").ap()
    wp_ap = nc.dram_tensor("wp", [128, 2048], BF, kind="ExternalInput").ap()
    bqk_ap = nc.dram_tensor("bqk", [128, 4], F32, kind="ExternalInput").ap()
    return xt_ap, wqk_ap, wv_ap, wp_ap, bqk_ap


def _build_common(opts, body_wrap):
    opts = {**DEFAULT_OPTS, **(opts or {})}
    nc = bacc.Bacc("TRN2", target_bir_lowering=False, debug=False)
    aps = _make_aps(nc)
    out_dt = BF if opts["out_dt"] == "bf16" else F32
    outp_ap = nc.dram_tensor("outp", [T, C], out_dt, kind="ExternalOutput").ap()
    aps = aps + (outp_ap,)
    with tile.TileContext(nc) as tc:
        with (
            tc.tile_pool(name="const", bufs=1) as const_pool,
            tc.tile_pool(name="qkT", bufs=1) as qkT_pool,
            tc.tile_pool(name="v", bufs=1) as v_pool,
            tc.tile_pool(name="yT", bufs=1) as yT_pool,
            tc.tile_pool(name="es", bufs=1) as es_pool,
            tc.tile_pool(name="outsb", bufs=1) as out_pool,
            tc.tile_pool(name="lrow", bufs=1) as lrow_pool,
            tc.tile_pool(name="bs", bufs=1) as bs_pool,
            tc.tile_pool(name="ps", bufs=1, space="PSUM") as psum_pool,
        ):
            pools = (const_pool, qkT_pool, v_pool, yT_pool, es_pool,
                     out_pool, lrow_pool, bs_pool, psum_pool)
            body_wrap(nc, tc, pools, aps, opts)
    nc.compile()
    return nc


def build(reps=1, opts=None):
    opts_full = {**DEFAULT_OPTS, **(opts or {})}
    def wrap(nc, tc, pools, aps, o):
        for _ in range(reps):
            emit_body(nc, tc, pools, aps, o)
    return _build_common(opts, wrap)


def build_looped(n_iters, opts=None):
    """Body wrapped in a hardware For_i loop, for wall-clock slope timing."""
    opts_full = {**DEFAULT_OPTS, **(opts or {})}
    def wrap(nc, tc, pools, aps, o):
        hints = tuple(getattr(mybir.EngineType, e) if isinstance(e, str) else e
                      for e in o["hint_engines"])
        with tc.For_i(0, n_iters, 1,
                      hint_engines=hints,
                      staggered_reset=o["staggered_reset"]):
            emit_body(nc, tc, pools, aps, o)
    return _build_common(opts, wrap)


_NC_CACHE = {}


def _get_nc(reps=1, opts=None):
    key = (reps, tuple(sorted((opts or {}).items())))
    if key not in _NC_CACHE:
        _NC_CACHE[key] = build(reps, opts)
    return _NC_CACHE[key]


def make_in_maps(x, w_attn, b_attn, w_proj):
    x = np.asarray(x, np.float32)
    w_attn = np.asarray(w_attn, np.float32)
    b_attn = np.asarray(b_attn, np.float32)
    in_maps = []
    # xt: [4 mc][128 p][8 cc * 512 m] — exact SBUF image per m-chunk tile
    xt_b = []
    for b in range(B):
        xT = x[b].T                                             # [C, T]
        x4 = xT.reshape(CC, 128, MC, 512).transpose(2, 1, 0, 3)  # [mc, p, cc, m]
        xt_b.append(np.ascontiguousarray(x4).astype(BF16).reshape(512, 4096))
    for c in range(N_CORES):
        b, g = divmod(c, GROUPS)
        h0 = HPC * g
        qs, ks = h0 * D, C + h0 * D
        # wqk sbuf image: [4 pp][128 p][8 cc * 128 n], stored [2x128 p, 2048]
        wqk_cols = np.concatenate([
            0.125 * w_attn[:, qs:qs + 128], 0.125 * w_attn[:, qs + 128:qs + 256],
            w_attn[:, ks:ks + 128], w_attn[:, ks + 128:ks + 256]], axis=1)  # [C, 512]
        wqk4 = wqk_cols.reshape(CC, 128, 4, 128).transpose(2, 1, 0, 3)      # [pp, p, cc, n]
        wqk = np.ascontiguousarray(wqk4).astype(BF16).reshape(512, 1024)
        # wv sbuf image: [128 p][8 cc * 256 n]
        wv4 = w_attn[:, 2 * C + g * 256: 2 * C + (g + 1) * 256] \
            .reshape(CC, 128, 256).transpose(1, 0, 2)
        wv = np.ascontiguousarray(wv4).astype(BF16).reshape(128, 2048)
        # wp sbuf image: [128 p][2 pp2 * 1024 n]
        wp4 = np.asarray(w_proj, np.float32)[g * 256:(g + 1) * 256, :] \
            .reshape(2, 128, 1024).transpose(1, 0, 2)
        wp = np.ascontiguousarray(wp4).astype(BF16).reshape(128, 2048)
        bqk = np.stack([
            0.125 * b_attn[qs:qs + 128], 0.125 * b_attn[qs + 128:qs + 256],
            b_attn[ks:ks + 128], b_attn[ks + 128:ks + 256]], axis=1).astype(np.float32)
        in_maps.append({"xt": xt_b[b], "wqk": wqk, "wv": wv, "wp": wp,
                        "bqk": np.ascontiguousarray(bqk)})
    return in_maps


def assemble_output(results, b_attn, w_proj, b_proj):
    b_attn = np.asarray(b_attn, np.float32)
    w_proj = np.asarray(w_proj, np.float32)
    b_proj = np.asarray(b_proj, np.float32)
    extra = b_attn[2 * C:] @ w_proj + b_proj  # v-bias flows through softmax as +bv
    out = np.empty((B, T, C), np.float32)
    for b in range(B):
        acc = results[4 * b]["outp"].astype(np.float32)
        for g in range(1, GROUPS):
            acc = acc + results[4 * b + g]["outp"].astype(np.float32)
        out[b] = acc + extra
    return out


def kernel(x, w_attn, b_attn, w_proj, b_proj):
    nc = _get_nc(reps=1)
    in_maps = make_in_maps(x, w_attn, b_attn, w_proj)
    res = run_bass_kernel_spmd(nc, in_maps, list(range(N_CORES)))
    return assemble_output(res.results, b_attn, w_proj, b_proj)
